# revision 28
# baseline (speedup 1.0000x reference)
"""Trainium2 Bass kernel for DiamondLayer.

Computes out[b, d] = mean(x[b, d:d+16, d+17:d+33]) for d in [0, 2016):
16x16 mean-pool windows sliding along the diagonal of each 2048x2048 matrix.

Sharding: pure data parallel over batch - 32 batches -> 8 cores x 4 batches.

Per-core kernel (raw bacc, no Tile). Partition p holds band rows
[16p, 16p+16), each row the 32 cols [r+2, r+34) (one 128B DMA run per
row; ~1/64 of the matrix). Structure per batch:

  A-band DMA (sync queue): rows 0..14 for partitions 0..127 - using 128
  partitions folds the tail partition's rows in (partition 127 reads
  in-bounds garbage that is never consumed; 127-partition DMAs land
  every packet on a single SDMA engine, and a separate tail DMA would
  cost another ~0.7us of issue time - DMA instruction issue occupies the
  sequencer ~0.6-1.6us each, a first-class budget)
  B-band DMA (scalar queue): row 15, partitions 0..125; plus one merged
  4-batch DMA for the tail partition's row 15
  -> DVE prefix scans: rows 0..14 (pp[1..480)) and row 15 (pp[481..513))
     independently - windows never span rows, so the prefixes never mix
     and the halo depends only on the rows 0..14 part
  -> sub C[16t+m] = P[32t+m+16] - P[32t+m] (bf16; gpsimd, last batch
     inline on DVE to skip the cross-engine hop on the critical chain)
  -> halo (scalar queue): C[q+1, 0:240] -> C[q, 256:496] so partition q
     holds window sums for all 31 rows its diamonds touch (C[16tau+m] is
     injective: 16u+15s collides only at |u-u'|=15, out of range)
  -> DVE reduce out[16q+u] = sum_s C[q, 15+16u+15s], scale 1/256
  -> one merged out DMA (sync queue).

Engines pipeline under relaxed ordering, so every RAW pair is
semaphore-guarded, same-engine included. A dummy gpsimd op at start
absorbs the ~0.7us first-op ucode warmup.
"""

import os
import sys

import numpy as np

for _p in ("/opt/trn_rl_repo",):
    if _p not in sys.path:
        sys.path.insert(0, _p)

B_FULL = 32
N_CORES = 8
B_PER_CORE = B_FULL // N_CORES  # 4
MAT = 2048
ND = MAT - 32  # 2016
NQ = ND // 16  # 126  (diamond groups)
NP = NQ + 1  # 127  (partitions holding band rows)
ROW_STRIDE = MAT + 1  # 2049
MAT_ELEMS = MAT * MAT
PW = 528  # band / prefix cols per batch (needs 513)
PW4 = 4 * PW
CW = 512  # window-sum cols per partition (uses 496)

LAST_EXEC_TIME_NS = None
LAST_TRACE_DIR = None
_COMPILED = None


def _build():
    import concourse.bass as bass
    import concourse.bacc as bacc
    from concourse import mybir
    from contextlib import ExitStack

    f32 = mybir.dt.float32
    bf16 = mybir.dt.bfloat16
    add = mybir.AluOpType.add
    sub_op = mybir.AluOpType.subtract
    bypass = mybir.AluOpType.bypass
    X = mybir.AxisListType.X

    nc = bacc.Bacc("TRN2", target_bir_lowering=False, debug=False)
    x = nc.dram_tensor("x", [B_PER_CORE, MAT, MAT], f32, kind="ExternalInput")
    y = nc.dram_tensor("y", [B_PER_CORE, ND], f32, kind="ExternalOutput")

    def v(t, off, pat):
        return bass.AP(t, off, pat)

    with ExitStack() as ctx:
        B = B_PER_CORE
        e = ctx.enter_context
        bta = e(nc.sbuf_tensor("bta", [128, PW4], f32))
        pps = [e(nc.sbuf_tensor(f"pp{i}", [NP, PW], f32)) for i in range(B)]
        cs = [e(nc.sbuf_tensor(f"c{i}", [NP, CW], bf16)) for i in range(B)]
        yvs = [e(nc.sbuf_tensor(f"yv{i}", [NQ, 16], f32)) for i in range(B)]
        yoa = e(nc.sbuf_tensor("yoa", [NQ, 64], f32))
        bsemA = [e(nc.semaphore(f"bsa{i}")) for i in range(B)]
        bsemB = [e(nc.semaphore(f"bsb{i}")) for i in range(B)]
        tsem = e(nc.semaphore("tsem"))
        hsem = [e(nc.semaphore(f"hsem{i}")) for i in range(B)]
        vscanA = e(nc.semaphore("vscanA"))
        vscanB = e(nc.semaphore("vscanB"))
        initsem = e(nc.semaphore("initsem"))
        subAsem = e(nc.semaphore("subAsem"))
        subBsem = e(nc.semaphore("subBsem"))
        subLasem = e(nc.semaphore("subLasem"))
        subLbsem = e(nc.semaphore("subLbsem"))
        redsem = e(nc.semaphore("redsem"))
        scalesem = e(nc.semaphore("scalesem"))
        outsem = e(nc.semaphore("outsem"))
        block = e(nc.Block(no_gpsimd_drain=True))

        LB = B - 1  # last-processed batch: subs inline on DVE

        def subs(b, eng, eng_ns, sa, sb_, wait_init):
            # C[16t+m] = P[32t+m+16] - P[32t+m], rows 0..14 (A) and
            # row 15 from its own prefix (B); C stored bf16
            eng.wait_ge(vscanA, b + 1)
            if wait_init:
                eng.wait_ge(initsem, 1)
            eng_ns.tensor_tensor(
                out=v(cs[b], 0, [[CW, NP], [16, 15], [1, 16]]),
                in0=v(pps[b], 16, [[PW, NP], [32, 15], [1, 16]]),
                in1=v(pps[b], 0, [[PW, NP], [32, 15], [1, 16]]),
                op=sub_op,
            ).then_inc(sa, 1)
            eng.wait_ge(vscanB, b + 1)
            eng_ns.tensor_tensor(
                out=v(cs[b], 240, [[CW, NP], [1, 16]]),
                in0=v(pps[b], 496, [[PW, NP], [1, 16]]),
                in1=v(pps[b], 480, [[PW, NP], [1, 16]]),
                op=sub_op,
            ).then_inc(sb_, 1)

        @block.sync
        def _(sync):
            for b in range(B):
                # band rows 0..14, partitions 0..127 (127's rows 2032..
                # 2046 are in-bounds garbage, never consumed)
                sync.dma_start(
                    v(bta, b * PW + 1, [[PW4, 128], [32, 15], [1, 32]]),
                    bass.AP(
                        x,
                        b * MAT_ELEMS + 2,
                        [[16 * ROW_STRIDE, 128], [ROW_STRIDE, 15], [1, 32]],
                    ),
                ).then_inc(bsemA[b], 16)
            # one merged out DMA for all batches
            sync.wait_ge(scalesem, B)
            sync.dma_start(
                bass.AP(y, 0, [[16, NQ], [ND, B], [1, 16]]),
                v(yoa, 0, [[64, NQ], [16, B], [1, 16]]),
            ).then_inc(outsem, 16)
            sync.wait_ge(outsem, 16)

        @block.vector
        def _(vector):
            def scanA(b):
                # P[f] = prefix of band rows 0..14; writes pp[1..480),
                # pp[0] and pp[480] stay memset-0
                vector.wait_ge(bsemA[b], 16)
                nc.vector.tensor_tensor_scan(
                    out=v(pps[b], 1, [[PW, NP], [1, 479]]),
                    data0=v(bta, b * PW + 1, [[PW4, NP], [1, 479]]),
                    data1=v(bta, b * PW + 1, [[PW4, NP], [1, 479]]),
                    initial=0.0,
                    op0=add,
                    op1=bypass,
                ).then_inc(vscanA, 1)

            def scanB(b):
                # independent prefix of band row 15 into pp[481..513)
                vector.wait_ge(bsemB[b], 16)
                vector.wait_ge(tsem, 16)
                nc.vector.tensor_tensor_scan(
                    out=v(pps[b], 481, [[PW, NP], [1, 32]]),
                    data0=v(bta, b * PW + 481, [[PW4, NP], [1, 32]]),
                    data1=v(bta, b * PW + 481, [[PW4, NP], [1, 32]]),
                    initial=0.0,
                    op0=add,
                    op1=bypass,
                ).then_inc(vscanB, 1)

            def red(b):
                # out[16q+u] = sum_s C[q, 15+16u+15s]
                vector.wait_ge(hsem[b], 16)
                if b == LB:
                    vector.wait_ge(subLbsem, 1)
                else:
                    vector.wait_ge(subBsem, b + 1)
                nc.vector.reduce_sum(
                    out=v(yvs[b], 0, [[16, NQ], [1, 16]]),
                    in_=v(cs[b], 15, [[CW, NQ], [16, 16], [15, 16]]),
                    axis=X,
                ).then_inc(redsem, 1)
                vector.wait_ge(redsem, b + 1)
                nc.vector.tensor_scalar_mul(
                    v(yoa, b * 16, [[64, NQ], [1, 16]]),
                    v(yvs[b], 0, [[16, NQ], [1, 16]]),
                    1.0 / 256.0,
                ).then_inc(scalesem, 1)

            for b in range(B):
                nc.vector.memset(pps[b][0:NP, 0:1], 0.0)
                m = nc.vector.memset(v(pps[b], 480, [[PW, NP], [1, 1]]), 0.0)
                if b == B - 1:
                    m.then_inc(initsem, 1)
            for b in range(B):
                scanA(b)
                scanB(b)
            subs(LB, vector, nc.vector, subLasem, subLbsem, False)
            for b in range(B):
                red(b)

        @block.gpsimd
        def _(gpsimd):
            # dummy op: absorbs the ~0.7us first-op ucode warmup while
            # the band DMAs are still draining
            nc.gpsimd.tensor_scalar_mul(
                v(pps[0], 514, [[PW, 1], [1, 8]]),
                v(pps[0], 514, [[PW, 1], [1, 8]]),
                0.0,
            )
            for b in range(B - 1):
                subs(b, gpsimd, nc.gpsimd, subAsem, subBsem, b == 0)

        @block.scalar
        def _(scalar):
            for b in range(B):
                # band row 15, partitions 0..125
                scalar.dma_start(
                    v(bta, b * PW + 481, [[PW4, NQ], [1, 32]]),
                    bass.AP(
                        x,
                        b * MAT_ELEMS + 2 + 15 * ROW_STRIDE,
                        [[16 * ROW_STRIDE, NQ], [1, 32]],
                    ),
                ).then_inc(bsemB[b], 16)
            # tail partition's row 15 (row 2031), all batches in one DMA
            scalar.dma_start(
                v(bta, NQ * PW4 + 481, [[PW4, 1], [PW, B], [1, 32]]),
                bass.AP(
                    x,
                    2 + (NQ * 16 + 15) * ROW_STRIDE,
                    [[16 * ROW_STRIDE, 1], [MAT_ELEMS, B], [1, 32]],
                ),
            ).then_inc(tsem, 16)

            def halo(b):
                # C[q, 256+f] = C[q+1, f], f in [0, 240): only rows 0..14
                # of the neighbor are ever needed, i.e. subA alone
                if b == LB:
                    scalar.wait_ge(subLasem, 1)
                else:
                    scalar.wait_ge(subAsem, b + 1)
                scalar.dma_start(
                    v(cs[b], 256, [[CW, NQ], [1, 240]]),
                    v(cs[b], CW, [[CW, NQ], [1, 240]]),
                ).then_inc(hsem[b], 16)

            for b in range(B):
                halo(b)

    nc.compile()
    return nc


def _get_compiled():
    global _COMPILED
    if _COMPILED is None:
        _COMPILED = _build()
    return _COMPILED


def kernel(x: np.ndarray) -> np.ndarray:
    global LAST_EXEC_TIME_NS, LAST_TRACE_DIR
    from concourse.bass_utils import run_bass_kernel_spmd

    x = np.ascontiguousarray(np.asarray(x), dtype=np.float32)
    assert x.shape == (B_FULL, MAT, MAT), x.shape

    nc = _get_compiled()
    in_maps = [
        {"x": x[i * B_PER_CORE : (i + 1) * B_PER_CORE]} for i in range(N_CORES)
    ]
    trace = bool(int(os.environ.get("KERNEL_TRACE", "0")))
    kwargs = {}
    if trace:
        # test-only: keep NTFF artifacts local instead of uploading
        from concourse import bass_utils as _bu
        import tempfile

        _bu.upload_artifacts = lambda tmpdir: tmpdir
        LAST_TRACE_DIR = tempfile.mkdtemp(prefix="ktrace_")
        kwargs["tmpdir"] = LAST_TRACE_DIR
    res = run_bass_kernel_spmd(
        nc, in_maps, core_ids=list(range(N_CORES)), trace=trace, **kwargs
    )
    LAST_EXEC_TIME_NS = res.exec_time_ns
    out = np.concatenate([res.results[i]["y"] for i in range(N_CORES)], axis=0)
    return out.astype(np.float32)


# revision 29
# speedup vs baseline: 1.0629x; 1.0629x over previous
"""Trainium2 Bass kernel for DiamondLayer.

Computes out[b, d] = mean(x[b, d:d+16, d+17:d+33]) for d in [0, 2016):
16x16 mean-pool windows sliding along the diagonal of each 2048x2048 matrix.

Sharding: pure data parallel over batch - 32 batches -> 8 cores x 4 batches.

Per-core kernel (raw bacc, no Tile). Partition p holds band rows
[16p, 16p+16), each row the 32 cols [r+2, r+34) (one 128B DMA run per
row; ~1/64 of the matrix). Structure per batch:

  A-band DMA (sync queue): rows 0..14 for partitions 0..127 - using 128
  partitions folds the tail partition's rows in (partition 127 reads
  in-bounds garbage that is never consumed; 127-partition DMAs land
  every packet on a single SDMA engine, and a separate tail DMA would
  cost another ~0.7us of issue time - DMA instruction issue occupies the
  sequencer ~0.6-1.6us each, a first-class budget)
  B-band DMA (scalar queue): row 15, partitions 0..125; plus one merged
  4-batch DMA for the tail partition's row 15
  -> DVE prefix scans: rows 0..14 (pp[1..480)) and row 15 (pp[481..513))
     independently - windows never span rows, so the prefixes never mix
     and the halo depends only on the rows 0..14 part
  -> sub C[16t+m] = P[32t+m+16] - P[32t+m] (bf16; gpsimd, last batch
     inline on DVE to skip the cross-engine hop on the critical chain)
  -> halo (scalar queue): C[q+1, 0:240] -> C[q, 256:496] so partition q
     holds window sums for all 31 rows its diamonds touch (C[16tau+m] is
     injective: 16u+15s collides only at |u-u'|=15, out of range)
  -> DVE reduce out[16q+u] = sum_s C[q, 15+16u+15s], scale 1/256
  -> one merged out DMA (sync queue).

Engines pipeline under relaxed ordering, so every RAW pair is
semaphore-guarded, same-engine included. A dummy gpsimd op at start
absorbs the ~0.7us first-op ucode warmup.
"""

import os
import sys

import numpy as np

for _p in ("/opt/trn_rl_repo",):
    if _p not in sys.path:
        sys.path.insert(0, _p)

B_FULL = 32
N_CORES = 8
B_PER_CORE = B_FULL // N_CORES  # 4
MAT = 2048
ND = MAT - 32  # 2016
NQ = ND // 16  # 126  (diamond groups)
NP = NQ + 1  # 127  (partitions holding band rows)
ROW_STRIDE = MAT + 1  # 2049
MAT_ELEMS = MAT * MAT
PW = 528  # band / prefix cols per batch (needs 513)
PW4 = 4 * PW
CW = 512  # window-sum cols per partition (uses 496)

LAST_EXEC_TIME_NS = None
LAST_TRACE_DIR = None
_COMPILED = None


def _build():
    import concourse.bass as bass
    import concourse.bacc as bacc
    from concourse import mybir
    from contextlib import ExitStack

    f32 = mybir.dt.float32
    bf16 = mybir.dt.bfloat16
    add = mybir.AluOpType.add
    sub_op = mybir.AluOpType.subtract
    bypass = mybir.AluOpType.bypass
    X = mybir.AxisListType.X

    nc = bacc.Bacc("TRN2", target_bir_lowering=False, debug=False)
    x = nc.dram_tensor("x", [B_PER_CORE, MAT, MAT], f32, kind="ExternalInput")
    y = nc.dram_tensor("y", [B_PER_CORE, ND], f32, kind="ExternalOutput")

    def v(t, off, pat):
        return bass.AP(t, off, pat)

    with ExitStack() as ctx:
        B = B_PER_CORE
        e = ctx.enter_context
        bta = e(nc.sbuf_tensor("bta", [128, PW4], f32))
        pps = [e(nc.sbuf_tensor(f"pp{i}", [NP, PW], f32)) for i in range(B)]
        cs = [e(nc.sbuf_tensor(f"c{i}", [NP, CW], f32)) for i in range(B)]
        yvs = [e(nc.sbuf_tensor(f"yv{i}", [NQ, 16], f32)) for i in range(B)]
        yoa = e(nc.sbuf_tensor("yoa", [NQ, 64], f32))
        bsemA = [e(nc.semaphore(f"bsa{i}")) for i in range(B)]
        bsemB = [e(nc.semaphore(f"bsb{i}")) for i in range(B)]
        tsem = [e(nc.semaphore(f"ts{i}")) for i in range(B)]
        hsem = [e(nc.semaphore(f"hsem{i}")) for i in range(B)]
        vscanA = e(nc.semaphore("vscanA"))
        vscanB = e(nc.semaphore("vscanB"))
        initsem = e(nc.semaphore("initsem"))
        subAsem = e(nc.semaphore("subAsem"))
        subBsem = e(nc.semaphore("subBsem"))
        subLasem = e(nc.semaphore("subLasem"))
        subLbsem = e(nc.semaphore("subLbsem"))
        redsem = e(nc.semaphore("redsem"))
        scalesem = e(nc.semaphore("scalesem"))
        outsem = e(nc.semaphore("outsem"))
        block = e(nc.Block(no_gpsimd_drain=True))

        LB = B - 1  # last-processed batch: subs inline on DVE

        def subs(b, eng, eng_ns, sa, sb_, wait_init):
            # C[16t+m] = P[32t+m+16] - P[32t+m], rows 0..14 (A) and
            # row 15 from its own prefix (B); C stored bf16
            eng.wait_ge(vscanA, b + 1)
            if wait_init:
                eng.wait_ge(initsem, 1)
            eng_ns.tensor_tensor(
                out=v(cs[b], 0, [[CW, NP], [16, 15], [1, 16]]),
                in0=v(pps[b], 16, [[PW, NP], [32, 15], [1, 16]]),
                in1=v(pps[b], 0, [[PW, NP], [32, 15], [1, 16]]),
                op=sub_op,
            ).then_inc(sa, 1)
            eng.wait_ge(vscanB, b + 1)
            eng_ns.tensor_tensor(
                out=v(cs[b], 240, [[CW, NP], [1, 16]]),
                in0=v(pps[b], 496, [[PW, NP], [1, 16]]),
                in1=v(pps[b], 480, [[PW, NP], [1, 16]]),
                op=sub_op,
            ).then_inc(sb_, 1)

        @block.sync
        def _(sync):
            for b in range(B):
                # band rows 0..14, partitions 0..127 (127's rows 2032..
                # 2046 are in-bounds garbage, never consumed)
                sync.dma_start(
                    v(bta, b * PW + 1, [[PW4, 128], [32, 15], [1, 32]]),
                    bass.AP(
                        x,
                        b * MAT_ELEMS + 2,
                        [[16 * ROW_STRIDE, 128], [ROW_STRIDE, 15], [1, 32]],
                    ),
                ).then_inc(bsemA[b], 16)
            # one merged out DMA for all batches
            sync.wait_ge(scalesem, B)
            sync.dma_start(
                bass.AP(y, 0, [[16, NQ], [ND, B], [1, 16]]),
                v(yoa, 0, [[64, NQ], [16, B], [1, 16]]),
            ).then_inc(outsem, 16)
            sync.wait_ge(outsem, 16)

        @block.vector
        def _(vector):
            def scanA(b):
                # P[f] = prefix of band rows 0..14; writes pp[1..480),
                # pp[0] and pp[480] stay memset-0
                vector.wait_ge(bsemA[b], 16)
                nc.vector.tensor_tensor_scan(
                    out=v(pps[b], 1, [[PW, NP], [1, 479]]),
                    data0=v(bta, b * PW + 1, [[PW4, NP], [1, 479]]),
                    data1=v(bta, b * PW + 1, [[PW4, NP], [1, 479]]),
                    initial=0.0,
                    op0=add,
                    op1=bypass,
                ).then_inc(vscanA, 1)

            def scanB(b):
                # independent prefix of band row 15 into pp[481..513)
                vector.wait_ge(bsemB[b], 16)
                vector.wait_ge(tsem[b], 16)
                nc.vector.tensor_tensor_scan(
                    out=v(pps[b], 481, [[PW, NP], [1, 32]]),
                    data0=v(bta, b * PW + 481, [[PW4, NP], [1, 32]]),
                    data1=v(bta, b * PW + 481, [[PW4, NP], [1, 32]]),
                    initial=0.0,
                    op0=add,
                    op1=bypass,
                ).then_inc(vscanB, 1)

            def red(b):
                # out[16q+u] = sum_s C[q, 15+16u+15s]
                vector.wait_ge(hsem[b], 16)
                if b == LB:
                    vector.wait_ge(subLbsem, 1)
                else:
                    vector.wait_ge(subBsem, b + 1)
                nc.vector.reduce_sum(
                    out=v(yvs[b], 0, [[16, NQ], [1, 16]]),
                    in_=v(cs[b], 15, [[CW, NQ], [16, 16], [15, 16]]),
                    axis=X,
                ).then_inc(redsem, 1)
                vector.wait_ge(redsem, b + 1)
                nc.vector.tensor_scalar_mul(
                    v(yoa, b * 16, [[64, NQ], [1, 16]]),
                    v(yvs[b], 0, [[16, NQ], [1, 16]]),
                    1.0 / 256.0,
                ).then_inc(scalesem, 1)

            for b in range(B):
                nc.vector.memset(pps[b][0:NP, 0:1], 0.0)
                m = nc.vector.memset(v(pps[b], 480, [[PW, NP], [1, 1]]), 0.0)
                if b == B - 1:
                    m.then_inc(initsem, 1)
            for b in range(B):
                scanA(b)
                scanB(b)
            subs(LB, vector, nc.vector, subLasem, subLbsem, False)
            for b in range(B):
                red(b)

        @block.gpsimd
        def _(gpsimd):
            # dummy op: absorbs the ~0.7us first-op ucode warmup while
            # the band DMAs are still draining
            nc.gpsimd.tensor_scalar_mul(
                v(pps[0], 514, [[PW, 1], [1, 8]]),
                v(pps[0], 514, [[PW, 1], [1, 8]]),
                0.0,
            )
            for b in range(B - 1):
                subs(b, gpsimd, nc.gpsimd, subAsem, subBsem, b == 0)

        @block.scalar
        def _(scalar):
            for b in range(B):
                # tail partition 126, all 16 rows (rows 0..14 duplicate
                # the A-band's write of the same values - benign)
                scalar.dma_start(
                    v(bta, NQ * PW4 + b * PW + 1, [[PW4, 1], [32, 16], [1, 32]]),
                    bass.AP(
                        x,
                        b * MAT_ELEMS + 2 + NQ * 16 * ROW_STRIDE,
                        [[16 * ROW_STRIDE, 1], [ROW_STRIDE, 16], [1, 32]],
                    ),
                ).then_inc(tsem[b], 16)
            for b in range(B):
                # band row 15, partitions 0..125
                scalar.dma_start(
                    v(bta, b * PW + 481, [[PW4, NQ], [1, 32]]),
                    bass.AP(
                        x,
                        b * MAT_ELEMS + 2 + 15 * ROW_STRIDE,
                        [[16 * ROW_STRIDE, NQ], [1, 32]],
                    ),
                ).then_inc(bsemB[b], 16)

            def halo(b):
                # C[q, 256+f] = C[q+1, f], f in [0, 240): only rows 0..14
                # of the neighbor are ever needed, i.e. subA alone
                if b == LB:
                    scalar.wait_ge(subLasem, 1)
                else:
                    scalar.wait_ge(subAsem, b + 1)
                scalar.dma_start(
                    v(cs[b], 256, [[CW, NQ], [1, 240]]),
                    v(cs[b], CW, [[CW, NQ], [1, 240]]),
                ).then_inc(hsem[b], 16)

            for b in range(B):
                halo(b)

    nc.compile()
    return nc


def _get_compiled():
    global _COMPILED
    if _COMPILED is None:
        _COMPILED = _build()
    return _COMPILED


def kernel(x: np.ndarray) -> np.ndarray:
    global LAST_EXEC_TIME_NS, LAST_TRACE_DIR
    from concourse.bass_utils import run_bass_kernel_spmd

    x = np.ascontiguousarray(np.asarray(x), dtype=np.float32)
    assert x.shape == (B_FULL, MAT, MAT), x.shape

    nc = _get_compiled()
    in_maps = [
        {"x": x[i * B_PER_CORE : (i + 1) * B_PER_CORE]} for i in range(N_CORES)
    ]
    trace = bool(int(os.environ.get("KERNEL_TRACE", "0")))
    kwargs = {}
    if trace:
        # test-only: keep NTFF artifacts local instead of uploading
        from concourse import bass_utils as _bu
        import tempfile

        _bu.upload_artifacts = lambda tmpdir: tmpdir
        LAST_TRACE_DIR = tempfile.mkdtemp(prefix="ktrace_")
        kwargs["tmpdir"] = LAST_TRACE_DIR
    res = run_bass_kernel_spmd(
        nc, in_maps, core_ids=list(range(N_CORES)), trace=trace, **kwargs
    )
    LAST_EXEC_TIME_NS = res.exec_time_ns
    out = np.concatenate([res.results[i]["y"] for i in range(N_CORES)], axis=0)
    return out.astype(np.float32)
